# revision 17
# baseline (speedup 1.0000x reference)
"""Trainium2 Bass kernel for nn_MultiHeadAttention_9912784519532.

MHA with relative position bias: b=2, n=2048, dim=512, heads=8, d_head=64,
rel table (2*512+1, 64).

Sharding: 16 (batch, head) pairs over 8 cores -> 2 heads of one batch per
core. Each core computes a partial output y_part = attn_out @ Wo_slice for
its 2 heads; host sums 4 partials per batch and adds bo.

Per-core algorithm (keys-on-partitions / transposed-attention orientation):
  qT/kT = W.T @ x.T via PE (2 heads packed on partitions 0-63 / 64-127)
  kT_past/kT_fut = kT + rel_emb[1024/0]  (folds the clipped far-field
    positional bias into the S^T matmul exactly)
  wER[n, c] = q[n] . rel_emb[clip(1152 - c, 0, 1024)]  (reversed+edge-padded
    relative projection table) -> DRAM scratch
  For each (head, q-chunk 1024, key-tile 128):
    Z^T = kT_variant.T @ qT  (class per 128-col block: past/window/future)
    window blocks: Z^T += transpose-matmul of diagonally-DMA'd wER tiles
      (pos[n, r] = wER[n, 640 - n + r] is a plain 2D-strided DRAM read)
    attnT = exp(0.125 * Z^T)  (ScalarE; logits are O(1), no max needed)
    outT_aug += [v | 1].T @ attnT  (row 64 accumulates the softmax denom)
  outT = outT_aug[:64] / outT_aug[64]; y_part = outT.T @ Wo_slice

All PE operands are bf16 (fp32 runs at 4 cycles/row on the PE, bf16 at 1);
accumulation stays fp32 in PSUM, and the final y is fp32.
"""

import numpy as np

HEADS = 8
D = 64
N = 2048
DIM = 512
WER = 1280  # width of padded/reversed rel projection table
P = 128

_cached = {}


def _build_program():
    import concourse.bass as bass
    import concourse.mybir as mybir
    import concourse.tile as tile
    from concourse import bacc

    f32 = mybir.dt.float32
    f32r = mybir.dt.float32r
    bf16 = mybir.dt.bfloat16
    AP = bass.AP

    nc = bacc.Bacc(
        "TRN2",
        target_bir_lowering=False,
        debug=False,
        enable_asserts=False,
        num_devices=8,
    )

    xT_d = nc.dram_tensor("xT", [DIM, N], bf16, kind="ExternalInput")
    wq_d = nc.dram_tensor("wq2", [DIM, P], bf16, kind="ExternalInput")
    wk_d = nc.dram_tensor("wk2", [DIM, P], bf16, kind="ExternalInput")
    wv_d = nc.dram_tensor("wv2", [DIM, P], bf16, kind="ExternalInput")
    wo_d = nc.dram_tensor("wo2", [P, DIM], bf16, kind="ExternalInput")
    relx_d = nc.dram_tensor("relx2", [P, WER], bf16, kind="ExternalInput")
    edge_d = nc.dram_tensor("edge2", [P, 2], f32, kind="ExternalInput")
    ident_d = nc.dram_tensor("ident", [P, P], bf16, kind="ExternalInput")
    sel_d = nc.dram_tensor("sel16", [16, 16 * 64], f32, kind="ExternalInput")
    y_d = nc.dram_tensor("y", [N, DIM], f32, kind="ExternalOutput")

    wer_d = [
        nc.dram_tensor(f"wer{h}", [N, WER], bf16, kind="Internal") for h in range(2)
    ]

    SCALE = float(D) ** -0.5
    NT = N // P  # 16 key tiles
    QW = 1024  # q-chunk width for the flash loop

    with tile.TileContext(nc) as tc:
        import contextlib

        ctx = contextlib.ExitStack()
        with ctx:
            const = ctx.enter_context(tc.tile_pool(name="const", bufs=1))
            big = ctx.enter_context(tc.tile_pool(name="big", bufs=1))
            cpool = ctx.enter_context(tc.tile_pool(name="copies", bufs=4))
            ppool = ctx.enter_context(tc.tile_pool(name="ps", bufs=2, space="PSUM"))
            opool = ctx.enter_context(tc.tile_pool(name="ot", bufs=4, space="PSUM"))
            apool = ctx.enter_context(tc.tile_pool(name="attn", bufs=3))
            wpool = ctx.enter_context(tc.tile_pool(name="win", bufs=8))
            spool = ctx.enter_context(tc.tile_pool(name="small", bufs=4))

            # ---- load constants / inputs ----
            xt_sb = big.tile([P, 4, N], bf16)
            nc.sync.dma_start(xt_sb[:], xT_d.ap().rearrange("(c p) n -> p c n", p=P))
            wq_sb = const.tile([P, 4, P], bf16)
            nc.sync.dma_start(wq_sb[:], wq_d.ap().rearrange("(c p) m -> p c m", p=P))
            wk_sb = const.tile([P, 4, P], bf16)
            nc.sync.dma_start(wk_sb[:], wk_d.ap().rearrange("(c p) m -> p c m", p=P))
            wv_sb = const.tile([P, 4, P], bf16)
            nc.sync.dma_start(wv_sb[:], wv_d.ap().rearrange("(c p) m -> p c m", p=P))
            wo_sb = const.tile([64, 2, DIM], bf16)
            nc.sync.dma_start(wo_sb[:], wo_d.ap().rearrange("(h p) m -> p h m", p=64))
            relx_sb = const.tile([P, WER], bf16)
            nc.sync.dma_start(relx_sb[:], relx_d.ap())
            edge_sb = const.tile([P, 2], f32)
            nc.sync.dma_start(edge_sb[:], edge_d.ap())
            ident_sb = const.tile([P, P], bf16)
            nc.sync.dma_start(ident_sb[:], ident_d.ap())
            sel_sb = const.tile([16, 16, 64], f32)
            nc.sync.dma_start(sel_sb[:], sel_d.ap())

            # ---- projections: qT2/kT2 (2 heads packed on partitions) ----
            qt2 = big.tile([P, N], bf16)
            kt2 = big.tile([P, N], bf16)
            for dst, wsb in ((qt2, wq_sb), (kt2, wk_sb)):
                for nch in range(4):
                    pt = ppool.tile([P, QW], f32, name="proj", tag="ps")
                    for cc in range(4):
                        nc.tensor.matmul(
                            pt[:, :512],
                            wsb[:, cc, :],
                            xt_sb[:, cc, nch * 512 : (nch + 1) * 512],
                            start=(cc == 0),
                            stop=(cc == 3),
                        )
                    nc.vector.tensor_copy(
                        dst[:, nch * 512 : (nch + 1) * 512], pt[:, :512]
                    )

            ktp = big.tile([P, N], bf16)
            ktf = big.tile([P, N], bf16)
            nc.vector.tensor_scalar_add(ktp[:], kt2[:], edge_sb[:, 0:1])
            nc.vector.tensor_scalar_add(ktf[:], kt2[:], edge_sb[:, 1:2])

            # ---- v (natural, keys on partitions), packed as [v | 1] ----
            v2 = big.tile([P, 2, NT, 65], bf16)
            nc.vector.memset(v2[:], 1.0)
            for kt in range(NT):
                pt = ppool.tile([P, QW], f32, name="vproj", tag="ps")
                for cc in range(4):
                    nc.tensor.matmul(
                        pt[:, :P],
                        xt_sb[:, cc, kt * P : (kt + 1) * P],
                        wv_sb[:, cc, :],
                        start=(cc == 0),
                        stop=(cc == 3),
                    )
                for h in range(2):
                    nc.vector.tensor_copy(
                        v2[:, h, kt, 0:64], pt[:, h * 64 : h * 64 + 64]
                    )

            # ---- wER tables -> DRAM (h0/h1 interleaved for row-group overlap) ----
            for qt in range(NT):
                wtiles = []
                for h in range(2):
                    hs = slice(h * 64, h * 64 + 64)
                    wtile = cpool.tile([P, WER], bf16, name=f"wer_sb{h}", tag="wer_sb")
                    for c0, cw in ((0, 512), (512, 512), (1024, 256)):
                        pt = ppool.tile([P, QW], f32, name="wer_ps", tag="ps")
                        nc.tensor.matmul(
                            pt[:, :cw],
                            qt2[hs, qt * P : (qt + 1) * P],
                            relx_sb[hs, c0 : c0 + cw],
                            start=True,
                            stop=True,
                        )
                        nc.vector.tensor_copy(wtile[:, c0 : c0 + cw], pt[:, :cw])
                    wtiles.append(wtile)
                for h in range(2):
                    nc.sync.dma_start(
                        wer_d[h].ap()[qt * P : (qt + 1) * P, :], wtiles[h][:]
                    )

            # ---- flash attention ----
            otn = big.tile([64, 2, N], bf16)  # normalized outT per head
            ostage = big.tile([65, 16, 512], f32)  # staged outT_aug per (h, qc, half)
            den = big.tile([16, 512], f32)
            nslot = 0
            for h in range(2):
                hs = slice(h * 64, h * 64 + 64)
                for qc in range(N // QW):
                    oth = [
                        opool.tile([65, 512], f32, name=f"outT{half}", tag="outT")
                        for half in range(2)
                    ]
                    for kt in range(NT):
                        kb = kt * P
                        zt = ppool.tile([P, QW], f32, name="zt", tag="ps")
                        cls = []
                        for j in range(QW // P):
                            dlt = qc * QW + j * P - kb
                            cls.append(
                                "p" if dlt >= 640 else ("f" if dlt <= -640 else "w")
                            )
                        # emit per 512-wide PSUM bank: runs never cross a bank,
                        # and the first matmul of each bank uses start=True
                        for half in range(QW // 512):
                            j0 = half * 4
                            runs = []
                            for j in range(j0, j0 + 4):
                                if runs and runs[-1][2] == cls[j]:
                                    runs[-1][1] += P
                                else:
                                    runs.append([j * P, P, cls[j]])
                            first = True
                            for s, wd, c in runs:
                                kvar = {"p": ktp, "f": ktf, "w": kt2}[c]
                                nc.tensor.matmul(
                                    zt[:, s : s + wd],
                                    kvar[hs, kb : kb + P],
                                    qt2[hs, qc * QW + s : qc * QW + s + wd],
                                    start=first,
                                    stop=False,
                                    skip_group_check=True,
                                )
                                first = False
                            wjs = [j for j in range(j0, j0 + 4) if cls[j] == "w"]
                            if wjs:
                                jw0, jw1 = wjs[0], wjs[-1]
                                ptile = wpool.tile(
                                    [P, 512], bf16, name="posT", tag="win"
                                )
                                for j in wjs:
                                    qb = qc * QW + j * P
                                    psrc = AP(
                                        tensor=wer_d[h],
                                        offset=qb * (WER - 1) + 640 + kb,
                                        ap=[[WER - 1, P], [1, P]],
                                    )
                                    eng = nc.sync if (j % 2 == 0) else nc.scalar
                                    eng.dma_start_transpose(
                                        ptile[:, (j - j0) * P : (j - j0 + 1) * P],
                                        psrc,
                                    )
                                nc.tensor.matmul(
                                    zt[:, jw0 * P : (jw1 + 1) * P],
                                    ident_sb[:],
                                    ptile[:, (jw0 - j0) * P : (jw1 - j0 + 1) * P],
                                    start=False,
                                    stop=False,
                                    skip_group_check=True,
                                )
                        at = apool.tile([P, QW], bf16, name="attnT")
                        nc.scalar.activation(
                            at[:], zt[:], mybir.ActivationFunctionType.Exp,
                            scale=SCALE,
                        )
                        for half in range(2):
                            nc.tensor.matmul(
                                oth[half][:],
                                v2[:, h, kt, :],
                                at[:, half * 512 : (half + 1) * 512],
                                start=(kt == 0),
                                stop=(kt == NT - 1),
                            )
                    for half in range(2):
                        nc.vector.tensor_copy(ostage[:, nslot, :], oth[half][:])
                        nc.sync.dma_start(
                            den[nslot : nslot + 1, :], ostage[64:65, nslot, :]
                        )
                        nslot += 1

            # batched softmax division: one reciprocal over all 16 denominators
            rden = big.tile([16, 512], f32)
            nc.vector.reciprocal(rden[:], den[:])
            nslot = 0
            for h in range(2):
                for qc in range(N // QW):
                    for half in range(2):
                        q0 = qc * QW + half * 512
                        rcb = opool.tile([64, 512], f32, name="recipb", tag="outT")
                        nc.tensor.matmul(
                            rcb[:],
                            sel_sb[:, nslot, :],
                            rden[:],
                            start=True, stop=True,
                        )
                        nc.vector.tensor_mul(
                            otn[:, h, q0 : q0 + 512],
                            ostage[0:64, nslot, :],
                            rcb[:],
                        )
                        nslot += 1

            # ---- output projection ----
            for nt in range(NT):
                pt = ppool.tile([P, QW], f32, name="yproj", tag="ps")
                for h in range(2):
                    nc.tensor.matmul(
                        pt[:, :512],
                        otn[:, h, nt * P : (nt + 1) * P],
                        wo_sb[:, h, :],
                        start=(h == 0), stop=(h == 1),
                    )
                yt = cpool.tile([P, 512], f32, name="y_sb")
                nc.vector.tensor_copy(yt[:], pt[:, :512])
                nc.sync.dma_start(y_d.ap()[nt * P : (nt + 1) * P, :], yt[:])

    nc.compile()
    return nc


def _host_prep(x, Wq, Wkv, Wo, rel_emb):
    """Build the 8 per-core input maps."""
    import ml_dtypes

    bf = ml_dtypes.bfloat16
    ident = np.eye(P, dtype=bf)
    sel16 = np.zeros((16, 16, 64), np.float32)
    for i in range(16):
        sel16[i, i, :] = 1.0
    sel16 = np.ascontiguousarray(sel16.reshape(16, 16 * 64))
    relX = rel_emb[np.clip(1152 - np.arange(WER), 0, 1024)].T
    relx2 = np.ascontiguousarray(np.concatenate([relX, relX], axis=0).astype(bf))
    edge = np.stack([rel_emb[1024], rel_emb[0]], axis=1)
    edge2 = np.ascontiguousarray(np.concatenate([edge, edge], axis=0).astype(np.float32))
    Wkv_r = Wkv.reshape(DIM, 2, HEADS, D)
    in_maps = []
    for core in range(8):
        b = core // 4
        h0 = 2 * (core % 4)
        in_maps.append(
            {
                "xT": np.ascontiguousarray(x[b].T.astype(bf)),
                "wq2": np.ascontiguousarray(Wq[:, h0 * D : (h0 + 2) * D].astype(bf)),
                "wk2": np.ascontiguousarray(
                    Wkv_r[:, 0, h0 : h0 + 2].reshape(DIM, 2 * D).astype(bf)
                ),
                "wv2": np.ascontiguousarray(
                    Wkv_r[:, 1, h0 : h0 + 2].reshape(DIM, 2 * D).astype(bf)
                ),
                "wo2": np.ascontiguousarray(
                    Wo[h0 * D : (h0 + 2) * D, :].astype(bf)
                ),
                "relx2": relx2,
                "edge2": edge2,
                "ident": ident,
                "sel16": sel16,
            }
        )
    return in_maps


def kernel(x, Wq, Wkv, Wo, bo, rel_emb, _want_trace=False):
    from concourse.bass_utils import run_bass_kernel_spmd

    x = np.asarray(x)
    if "nc" not in _cached:
        _cached["nc"] = _build_program()
    nc = _cached["nc"]
    in_maps = _host_prep(x, np.asarray(Wq), np.asarray(Wkv), np.asarray(Wo),
                         np.asarray(rel_emb))
    res = run_bass_kernel_spmd(
        nc, in_maps, core_ids=list(range(8)), trace=_want_trace
    )
    _cached["last_result"] = res
    y = np.zeros((2, N, DIM), np.float32)
    for core in range(8):
        y[core // 4] += res.results[core]["y"]
    y += np.asarray(bo).astype(np.float32)[None, None, :]
    return y


# revision 18
# speedup vs baseline: 1.4241x; 1.4241x over previous
"""Trainium2 Bass kernel for nn_MultiHeadAttention_9912784519532.

MHA with relative position bias: b=2, n=2048, dim=512, heads=8, d_head=64,
rel table (2*512+1, 64).

Sharding: 16 (batch, head) pairs over 8 cores -> 2 heads of one batch per
core. Each core computes a partial output y_part = attn_out @ Wo_slice for
its 2 heads; host sums 4 partials per batch and adds bo.

Per-core algorithm (keys-on-partitions / transposed-attention orientation):
  qT/kT = W.T @ x.T via PE (2 heads packed on partitions 0-63 / 64-127)
  kT_past/kT_fut = kT + rel_emb[1024/0]  (folds the clipped far-field
    positional bias into the S^T matmul exactly)
  wER[n, c] = q[n] . rel_emb[clip(1152 - c, 0, 1024)]  (reversed+edge-padded
    relative projection table) -> DRAM scratch
  For each (head, q-chunk 1024, key-tile 128):
    Z^T = kT_variant.T @ qT  (class per 128-col block: past/window/future)
    window blocks: Z^T += transpose-matmul of diagonally-DMA'd wER tiles
      (pos[n, r] = wER[n, 640 - n + r] is a plain 2D-strided DRAM read)
    attnT = exp(0.125 * Z^T)  (ScalarE; logits are O(1), no max needed)
    outT_aug += [v | 1].T @ attnT  (row 64 accumulates the softmax denom)
  outT = outT_aug[:64] / outT_aug[64]; y_part = outT.T @ Wo_slice

All PE operands are bf16 (fp32 runs at 4 cycles/row on the PE, bf16 at 1);
accumulation stays fp32 in PSUM, and the final y is fp32.
"""

import numpy as np

HEADS = 8
D = 64
N = 2048
DIM = 512
WER = 1280  # width of padded/reversed rel projection table
P = 128

_cached = {}


def _build_program():
    import concourse.bass as bass
    import concourse.mybir as mybir
    import concourse.tile as tile
    from concourse import bacc

    f32 = mybir.dt.float32
    f32r = mybir.dt.float32r
    bf16 = mybir.dt.bfloat16
    AP = bass.AP

    nc = bacc.Bacc(
        "TRN2",
        target_bir_lowering=False,
        debug=False,
        enable_asserts=False,
        num_devices=8,
    )

    xT_d = nc.dram_tensor("xT", [DIM, N], bf16, kind="ExternalInput")
    wq_d = nc.dram_tensor("wq2", [DIM, P], bf16, kind="ExternalInput")
    wk_d = nc.dram_tensor("wk2", [DIM, P], bf16, kind="ExternalInput")
    wv_d = nc.dram_tensor("wv2", [DIM, P], bf16, kind="ExternalInput")
    wo_d = nc.dram_tensor("wo2", [P, DIM], bf16, kind="ExternalInput")
    relx_d = nc.dram_tensor("relx2", [P, WER], bf16, kind="ExternalInput")
    edge_d = nc.dram_tensor("edge2", [P, 2], f32, kind="ExternalInput")
    ident_d = nc.dram_tensor("ident", [P, P], f32, kind="ExternalInput")
    sel_d = nc.dram_tensor("sel16", [16, 16 * 64], f32, kind="ExternalInput")
    y_d = nc.dram_tensor("y", [N, DIM], f32, kind="ExternalOutput")

    wer_d = [
        nc.dram_tensor(f"wer{h}", [N, WER], bf16, kind="Internal") for h in range(2)
    ]

    SCALE = float(D) ** -0.5
    NT = N // P  # 16 key tiles
    QW = 1024  # q-chunk width for the flash loop

    with tile.TileContext(nc) as tc:
        import contextlib

        ctx = contextlib.ExitStack()
        with ctx:
            const = ctx.enter_context(tc.tile_pool(name="const", bufs=1))
            big = ctx.enter_context(tc.tile_pool(name="big", bufs=1))
            cpool = ctx.enter_context(tc.tile_pool(name="copies", bufs=4))
            ppool = ctx.enter_context(tc.tile_pool(name="ps", bufs=2, space="PSUM"))
            opool = ctx.enter_context(tc.tile_pool(name="ot", bufs=4, space="PSUM"))
            apool = ctx.enter_context(tc.tile_pool(name="attn", bufs=3))
            wpool = ctx.enter_context(tc.tile_pool(name="win", bufs=8))
            spool = ctx.enter_context(tc.tile_pool(name="small", bufs=4))

            # ---- load constants / inputs ----
            xt_sb = big.tile([P, 4, N], bf16)
            nc.sync.dma_start(xt_sb[:], xT_d.ap().rearrange("(c p) n -> p c n", p=P))
            wq_sb = const.tile([P, 4, P], bf16)
            nc.sync.dma_start(wq_sb[:], wq_d.ap().rearrange("(c p) m -> p c m", p=P))
            wk_sb = const.tile([P, 4, P], bf16)
            nc.sync.dma_start(wk_sb[:], wk_d.ap().rearrange("(c p) m -> p c m", p=P))
            wv_sb = const.tile([P, 4, P], bf16)
            nc.sync.dma_start(wv_sb[:], wv_d.ap().rearrange("(c p) m -> p c m", p=P))
            wo_sb = const.tile([64, 2, DIM], bf16)
            nc.sync.dma_start(wo_sb[:], wo_d.ap().rearrange("(h p) m -> p h m", p=64))
            relx_sb = const.tile([P, WER], bf16)
            nc.sync.dma_start(relx_sb[:], relx_d.ap())
            edge_sb = const.tile([P, 2], f32)
            nc.sync.dma_start(edge_sb[:], edge_d.ap())
            ident_sb = const.tile([P, P], f32)
            nc.sync.dma_start(ident_sb[:], ident_d.ap())
            sel_sb = const.tile([16, 16, 64], f32)
            nc.sync.dma_start(sel_sb[:], sel_d.ap())

            # ---- projections: qT2/kT2 (2 heads packed on partitions) ----
            qt2 = big.tile([P, N], bf16)
            kt2 = big.tile([P, N], bf16)
            for dst, wsb in ((qt2, wq_sb), (kt2, wk_sb)):
                for nch in range(4):
                    pt = ppool.tile([P, QW], f32, name="proj", tag="ps")
                    for cc in range(4):
                        nc.tensor.matmul(
                            pt[:, :512],
                            wsb[:, cc, :],
                            xt_sb[:, cc, nch * 512 : (nch + 1) * 512],
                            start=(cc == 0),
                            stop=(cc == 3),
                        )
                    nc.vector.tensor_copy(
                        dst[:, nch * 512 : (nch + 1) * 512], pt[:, :512]
                    )

            ktp = big.tile([P, N], bf16)
            ktf = big.tile([P, N], bf16)
            nc.vector.tensor_scalar_add(ktp[:], kt2[:], edge_sb[:, 0:1])
            nc.vector.tensor_scalar_add(ktf[:], kt2[:], edge_sb[:, 1:2])

            # ---- v (natural, keys on partitions), packed as [v | 1] ----
            v2 = big.tile([P, 2, NT, 65], bf16)
            nc.vector.memset(v2[:], 1.0)
            for kt in range(NT):
                pt = ppool.tile([P, QW], f32, name="vproj", tag="ps")
                for cc in range(4):
                    nc.tensor.matmul(
                        pt[:, :P],
                        xt_sb[:, cc, kt * P : (kt + 1) * P],
                        wv_sb[:, cc, :],
                        start=(cc == 0),
                        stop=(cc == 3),
                    )
                for h in range(2):
                    nc.vector.tensor_copy(
                        v2[:, h, kt, 0:64], pt[:, h * 64 : h * 64 + 64]
                    )

            # ---- wER tables -> DRAM (h0/h1 interleaved for row-group overlap) ----
            for qt in range(NT):
                wtiles = []
                for h in range(2):
                    hs = slice(h * 64, h * 64 + 64)
                    wtile = cpool.tile([P, WER], bf16, name=f"wer_sb{h}", tag="wer_sb")
                    for c0, cw in ((0, 512), (512, 512), (1024, 256)):
                        pt = ppool.tile([P, QW], f32, name="wer_ps", tag="ps")
                        nc.tensor.matmul(
                            pt[:, :cw],
                            qt2[hs, qt * P : (qt + 1) * P],
                            relx_sb[hs, c0 : c0 + cw],
                            start=True,
                            stop=True,
                        )
                        nc.vector.tensor_copy(wtile[:, c0 : c0 + cw], pt[:, :cw])
                    wtiles.append(wtile)
                for h in range(2):
                    nc.sync.dma_start(
                        wer_d[h].ap()[qt * P : (qt + 1) * P, :], wtiles[h][:]
                    )

            # ---- flash attention ----
            otn = big.tile([64, 2, N], bf16)  # normalized outT per head
            ostage = big.tile([65, 16, 512], f32)  # staged outT_aug per (h, qc, half)
            den = big.tile([16, 512], f32)
            nslot = 0
            for h in range(2):
                hs = slice(h * 64, h * 64 + 64)
                for qc in range(N // QW):
                    wins = []
                    for j in range(QW // P):
                        qb = qc * QW + j * P
                        r0 = max(0, qb - 512)
                        r1 = min(N, qb + 640)
                        rw = r1 - r0
                        wt = wpool.tile([P, 1152], f32, name=f"win{j}", tag="win")
                        wsrc = AP(
                            tensor=wer_d[h],
                            offset=qb * (WER - 1) + 640 + r0,
                            ap=[[WER - 1, P], [1, rw]],
                        )
                        nc.gpsimd.dma_start(wt[:, :rw], wsrc)
                        wins.append((wt, r0))

                    oth = [
                        opool.tile([65, 512], f32, name=f"outT{half}", tag="outT")
                        for half in range(2)
                    ]
                    for kt in range(NT):
                        kb = kt * P
                        zt = ppool.tile([P, QW], f32, name="zt", tag="ps")
                        cls = []
                        for j in range(QW // P):
                            dlt = qc * QW + j * P - kb
                            cls.append(
                                "p" if dlt >= 640 else ("f" if dlt <= -640 else "w")
                            )
                        # emit per 512-wide PSUM bank: runs never cross a bank,
                        # and the first matmul of each bank uses start=True
                        for half in range(QW // 512):
                            j0 = half * 4
                            runs = []
                            for j in range(j0, j0 + 4):
                                if runs and runs[-1][2] == cls[j]:
                                    runs[-1][1] += P
                                else:
                                    runs.append([j * P, P, cls[j]])
                            first = True
                            for s, wd, c in runs:
                                kvar = {"p": ktp, "f": ktf, "w": kt2}[c]
                                nc.tensor.matmul(
                                    zt[:, s : s + wd],
                                    kvar[hs, kb : kb + P],
                                    qt2[hs, qc * QW + s : qc * QW + s + wd],
                                    start=first,
                                    stop=False,
                                    skip_group_check=True,
                                )
                                first = False
                            for j in range(j0, j0 + 4):
                                if cls[j] != "w":
                                    continue
                                wt, r0 = wins[j]
                                nc.tensor.matmul(
                                    zt[:, j * P : (j + 1) * P],
                                    wt[:, kb - r0 : kb - r0 + P],
                                    ident_sb[:],
                                    is_transpose=True,
                                    start=False,
                                    stop=False,
                                    skip_group_check=True,
                                )
                        at = apool.tile([P, QW], bf16, name="attnT")
                        nc.scalar.activation(
                            at[:], zt[:], mybir.ActivationFunctionType.Exp,
                            scale=SCALE,
                        )
                        for half in range(2):
                            nc.tensor.matmul(
                                oth[half][:],
                                v2[:, h, kt, :],
                                at[:, half * 512 : (half + 1) * 512],
                                start=(kt == 0),
                                stop=(kt == NT - 1),
                            )
                    for half in range(2):
                        nc.vector.tensor_copy(ostage[:, nslot, :], oth[half][:])
                        nc.sync.dma_start(
                            den[nslot : nslot + 1, :], ostage[64:65, nslot, :]
                        )
                        nslot += 1

            # batched softmax division: one reciprocal over all 16 denominators
            rden = big.tile([16, 512], f32)
            nc.vector.reciprocal(rden[:], den[:])
            nslot = 0
            for h in range(2):
                for qc in range(N // QW):
                    for half in range(2):
                        q0 = qc * QW + half * 512
                        rcb = opool.tile([64, 512], f32, name="recipb", tag="outT")
                        nc.tensor.matmul(
                            rcb[:],
                            sel_sb[:, nslot, :],
                            rden[:],
                            start=True, stop=True,
                        )
                        nc.vector.tensor_mul(
                            otn[:, h, q0 : q0 + 512],
                            ostage[0:64, nslot, :],
                            rcb[:],
                        )
                        nslot += 1

            # ---- output projection ----
            for nt in range(NT):
                pt = ppool.tile([P, QW], f32, name="yproj", tag="ps")
                for h in range(2):
                    nc.tensor.matmul(
                        pt[:, :512],
                        otn[:, h, nt * P : (nt + 1) * P],
                        wo_sb[:, h, :],
                        start=(h == 0), stop=(h == 1),
                    )
                yt = cpool.tile([P, 512], f32, name="y_sb")
                nc.vector.tensor_copy(yt[:], pt[:, :512])
                nc.sync.dma_start(y_d.ap()[nt * P : (nt + 1) * P, :], yt[:])

    nc.compile()
    return nc


def _host_prep(x, Wq, Wkv, Wo, rel_emb):
    """Build the 8 per-core input maps."""
    import ml_dtypes

    bf = ml_dtypes.bfloat16
    ident = np.eye(P, dtype=np.float32)
    sel16 = np.zeros((16, 16, 64), np.float32)
    for i in range(16):
        sel16[i, i, :] = 1.0
    sel16 = np.ascontiguousarray(sel16.reshape(16, 16 * 64))
    relX = rel_emb[np.clip(1152 - np.arange(WER), 0, 1024)].T
    relx2 = np.ascontiguousarray(np.concatenate([relX, relX], axis=0).astype(bf))
    edge = np.stack([rel_emb[1024], rel_emb[0]], axis=1)
    edge2 = np.ascontiguousarray(np.concatenate([edge, edge], axis=0).astype(np.float32))
    Wkv_r = Wkv.reshape(DIM, 2, HEADS, D)
    in_maps = []
    for core in range(8):
        b = core // 4
        h0 = 2 * (core % 4)
        in_maps.append(
            {
                "xT": np.ascontiguousarray(x[b].T.astype(bf)),
                "wq2": np.ascontiguousarray(Wq[:, h0 * D : (h0 + 2) * D].astype(bf)),
                "wk2": np.ascontiguousarray(
                    Wkv_r[:, 0, h0 : h0 + 2].reshape(DIM, 2 * D).astype(bf)
                ),
                "wv2": np.ascontiguousarray(
                    Wkv_r[:, 1, h0 : h0 + 2].reshape(DIM, 2 * D).astype(bf)
                ),
                "wo2": np.ascontiguousarray(
                    Wo[h0 * D : (h0 + 2) * D, :].astype(bf)
                ),
                "relx2": relx2,
                "edge2": edge2,
                "ident": ident,
                "sel16": sel16,
            }
        )
    return in_maps


def kernel(x, Wq, Wkv, Wo, bo, rel_emb, _want_trace=False):
    from concourse.bass_utils import run_bass_kernel_spmd

    x = np.asarray(x)
    if "nc" not in _cached:
        _cached["nc"] = _build_program()
    nc = _cached["nc"]
    in_maps = _host_prep(x, np.asarray(Wq), np.asarray(Wkv), np.asarray(Wo),
                         np.asarray(rel_emb))
    res = run_bass_kernel_spmd(
        nc, in_maps, core_ids=list(range(8)), trace=_want_trace
    )
    _cached["last_result"] = res
    y = np.zeros((2, N, DIM), np.float32)
    for core in range(8):
        y[core // 4] += res.results[core]["y"]
    y += np.asarray(bo).astype(np.float32)[None, None, :]
    return y
